# revision 30
# baseline (speedup 1.0000x reference)
"""Trainium2 Bass kernel for a binarized (1w/1a) BasicBlock — fp8 DoubleRow.

    a1 = sign(x);  y1 = BN(conv3x3(a1, binarize(w1))) + x;  x1 = maxout(y1)
    a2 = sign(x1); y2 = BN(conv3x3(a2, binarize(w2))) + x1; out = maxout(y2)

Data-parallel over batch (4 samples/core, 8 cores); exact binary math:
activations are +-1 (fp8e4, exact), weights are sign(+-1) fp8; each conv is
9 DoubleRow matmuls per (chunk, cout-block), contracting all 256 input
channels at once over contiguous padded-row runs (pad columns land in
unused psum columns).  conv_true = alpha_a*alpha[o]*(BB + q[o]*S1) with
q = beta/alpha; S1 (3x3 box of the channel sum) comes from 3 more DoubleRow
ones-matmuls (folding the kh taps) + 2 shifted adds.  The per-channel scale
folds into BN exactly by scaling BN_EPS per channel.  Batch-stat BN uses an
AllGather of per-core (count, mean, M2) triplets + bn_aggr on every core.

Scheduling (v2): sample-outer padded activation layout so the conv stream
starts as soon as the first sample pair is signed; one early warmup
AllGather absorbs the CC-ring establishment; apply1 only computes
t = BN(z)+x (into the dead cv space) and sign(t) -> a2, deferring the
maxout multiply (x1 = t*coef) into the conv2 window; round-2 conv runs
cout-block-major with a per-block stats AllGather so the second collective
overlaps conv2 and apply2(block0).
"""

import numpy as np
import ml_dtypes

import concourse.bass as bass
import concourse.bacc as bacc
import concourse.mybir as mybir
import concourse.tile as tile

N_CORES = 8
B, C, H, W = 32, 256, 28, 28
BPC = B // N_CORES            # samples per core
NBLK = 2                      # channel blocks of 128
HPAD, WPAD = 30, 30           # padded image in SBUF
PIX = H * W                   # 784
PPIX = HPAD * WPAD            # 900
SPLANE = 2 * PPIX             # both channel planes of one sample: 1800
NCHUNK = 2 * BPC              # 8 chunks of (sample, half-image)
HHALF = H // 2                # 14
CHUNK = HHALF * W             # 392 dense output elems per chunk
RUN = HHALF * WPAD            # 420: rhs run length / psum width per chunk
BN_EPS = 1e-5
NPRM = 24
GUARD = 32                    # fp8 guard elems around merged activation tile
SPAN = 4 * CHUNK              # 1568-wide apply spans (2 samples)
F32 = mybir.dt.float32
BF16 = mybir.dt.bfloat16
FP8 = mybir.dt.float8e4
AF = mybir.ActivationFunctionType
ALU = mybir.AluOpType
DR = mybir.MatmulPerfMode.DoubleRow


def _evac(nc, sc, ps, s1, sums, sumsqs, cv, prm, pcol, ci, oblk):
    """z = q[o]*S1 + BB from PSUM (strided: skip pad cols).  Stats come for
    free: the STT accumulates sum(z) on DVE; a Square pass on the otherwise
    idle ScalarE accumulates sum(z^2)."""
    psv = ps[:].rearrange("p (h w) -> p h w", h=HHALF)[:, :, 1:1 + W]
    s1v = s1[:].rearrange("p (h w) -> p h w", h=H)[
        :, (ci % 2) * HHALF:(ci % 2) * HHALF + HHALF, :]
    cvc = cv[oblk][:, ci * CHUNK:(ci + 1) * CHUNK]
    nc.vector.scalar_tensor_tensor(
        cvc.rearrange("p (h w) -> p h w", h=HHALF), s1v,
        prm[:, pcol['q'] + oblk:pcol['q'] + oblk + 1], psv,
        op0=ALU.mult, op1=ALU.add,
        accum_out=sums[:, oblk * NCHUNK + ci:oblk * NCHUNK + ci + 1])
    sqj = sc.tile([128, CHUNK], F32, tag="sqj", name="sqj", bufs=2)
    nc.scalar.activation(
        sqj[:], cvc, AF.Square,
        accum_out=sumsqs[:, oblk * NCHUNK + ci:oblk * NCHUNK + ci + 1])


def _s1_sample(nc, sc, psum, rnd, rhs_ap, ones3, b):
    """S1 (3x3 box of channel sums) for one sample: 3 kh-folding DoubleRow
    ones-matmuls per half + 2 shifted W-direction adds."""
    hs = sc.tile([128, 2 * RUN], F32, tag="hs", name="hs", bufs=2)
    for half in range(2):
        h0 = half * HHALF
        ps2 = psum.tile([128, RUN], F32, tag="ps2", name=f"ps2_{rnd}",
                        bufs=2)
        for kh in range(3):
            nc.tensor.matmul(ps2[:], ones3,
                             rhs_ap(b * SPLANE + (h0 + kh) * WPAD),
                             start=(kh == 0), stop=(kh == 2), perf_mode=DR)
        nc.scalar.copy(hs[:, half * RUN:half * RUN + RUN], ps2[:])
    hsv = hs[:].rearrange("p (h w) -> p h w", h=H)
    w3 = sc.tile([128, H * W], F32, tag="w3", name="w3", bufs=2)
    w3v = w3[:].rearrange("p (h w) -> p h w", h=H)
    nc.vector.tensor_add(w3v, hsv[:, :, 0:W], hsv[:, :, 1:1 + W])
    s1 = sc.tile([128, H * W], F32, tag="s1", name="s1", bufs=BPC)
    s1v = s1[:].rearrange("p (h w) -> p h w", h=H)
    nc.vector.tensor_add(s1v, w3v, hsv[:, :, 2:2 + W])
    return s1


def _chunk_mms(nc, pools, rnd, rhs_ap, wv, s1s, cv, prm, pcol, sums, sumsqs,
               ci, oblk):
    sbuf, psum, sc, dram = pools
    b, h0 = ci // 2, (ci % 2) * HHALF
    ps = psum.tile([128, RUN], F32, tag="ps", name=f"ps{rnd}", bufs=6)
    for k9 in range(9):
        kh, kw = k9 // 3, k9 % 3
        nc.tensor.matmul(
            ps[:], wv[:, k9, :, oblk * 128:(oblk + 1) * 128],
            rhs_ap(b * SPLANE + (h0 + kh) * WPAD + kw - 1),
            start=(k9 == 0), stop=(k9 == 8), perf_mode=DR)
    _evac(nc, sc, ps, s1s[b], sums, sumsqs, cv, prm, pcol, ci, oblk)


def _local_pair(nc, sums, sumsqs, oblk, dest, col):
    """Raw (sum, sumsq) over this oblk's 8 chunks -> dest[:, col:col+2].
    Only two reduces sit before the AllGather trigger."""
    nc.vector.reduce_sum(dest[:, col:col + 1],
                         sums[:, oblk * NCHUNK:(oblk + 1) * NCHUNK],
                         axis=mybir.AxisListType.X)
    nc.vector.reduce_sum(dest[:, col + 1:col + 2],
                         sumsqs[:, oblk * NCHUNK:(oblk + 1) * NCHUNK],
                         axis=mybir.AxisListType.X)


def _ag_start(nc, dram, rnd, pair, npair, tag):
    """DMA the local raw sums to DRAM and trigger the AllGather."""
    b_d = dram.tile([128, 2 * npair], F32, name=f"bd{rnd}{tag}")
    g_d = dram.tile([N_CORES, 128, 2 * npair], F32, name=f"gd{rnd}{tag}")
    nc.gpsimd.dma_start(b_d[:], pair[:])
    nc.gpsimd.collective_compute(
        "AllGather", ALU.bypass,
        replica_groups=[list(range(N_CORES))],
        ins=[b_d.opt()], outs=[g_d.opt()])
    return g_d


def _ag_gather(nc, sbuf, rnd, g_d, npair, tag):
    """Gather back r-major (contiguous per-rank j-tuples -> one fast DMA);
    the per-stat reduction later uses a strided AP."""
    gst = sbuf.tile([128, 2 * npair * N_CORES], F32, name=f"gst{rnd}{tag}")
    nc.sync.dma_start(
        gst[:].rearrange("p (r j) -> p r j", r=N_CORES),
        g_d[:].rearrange("r p j -> p r j"))
    return gst


def _ag_finish(nc, sbuf, rnd, gst, prm, pcol, oblk, jcol, tag):
    """Global mean/var from raw sums; per-channel scale/shift columns."""
    cnt = float(N_CORES * NCHUNK * CHUNK)
    mean = sbuf.tile([128, 1], F32, name=f"mean{rnd}{tag}")
    ms = sbuf.tile([128, 1], F32, name=f"ms{rnd}{tag}")
    m2 = sbuf.tile([128, 1], F32, name=f"m2{rnd}{tag}")
    var = sbuf.tile([128, 1], F32, name=f"var{rnd}{tag}")
    sd = sbuf.tile([128, 1], F32, name=f"sd{rnd}{tag}")
    inv = sbuf.tile([128, 1], F32, name=f"inv{rnd}{tag}")
    scale = sbuf.tile([128, 1], F32, name=f"scale{rnd}{tag}")
    tmp = sbuf.tile([128, 1], F32, name=f"tmp{rnd}{tag}")
    shift = sbuf.tile([128, 1], F32, name=f"shift{rnd}{tag}")
    gv = gst[:].rearrange("p (r j) -> p j r", r=N_CORES)
    nc.vector.reduce_sum(tmp[:], gv[:, jcol], axis=mybir.AxisListType.X)
    nc.vector.tensor_scalar_mul(mean[:], tmp[:], 1.0 / cnt)
    nc.vector.reduce_sum(tmp[:], gv[:, jcol + 1], axis=mybir.AxisListType.X)
    nc.vector.tensor_scalar_mul(ms[:], tmp[:], 1.0 / cnt)
    nc.vector.tensor_mul(m2[:], mean[:], mean[:])
    nc.vector.tensor_sub(var[:], ms[:], m2[:])
    nc.scalar.activation(sd[:], var[:], AF.Sqrt,
                         bias=prm[:, pcol['eps'] + oblk:
                                  pcol['eps'] + oblk + 1],
                         scale=1.0)
    nc.vector.reciprocal(inv[:], sd[:])
    nc.vector.tensor_mul(scale[:], inv[:],
                         prm[:, pcol['g'] + oblk:pcol['g'] + oblk + 1])
    nc.vector.tensor_mul(tmp[:], mean[:], scale[:])
    nc.vector.tensor_sub(shift[:],
                         prm[:, pcol['b'] + oblk:pcol['b'] + oblk + 1],
                         tmp[:])
    return scale, shift


def build():
    nc = bacc.Bacc("TRN2", target_bir_lowering=False, debug=False,
                   enable_asserts=True, num_devices=N_CORES)
    x_d = nc.dram_tensor("x", [BPC, C, H, W], F32, kind="ExternalInput")
    w1_d = nc.dram_tensor("w1t", [9, NBLK, 128, 256], FP8,
                          kind="ExternalInput")
    w2_d = nc.dram_tensor("w2t", [9, NBLK, 128, 256], FP8,
                          kind="ExternalInput")
    prm_d = nc.dram_tensor("prm", [128, NPRM], F32, kind="ExternalInput")
    out_d = nc.dram_tensor("out", [BPC, C, H, W], F32, kind="ExternalOutput")

    with tile.TileContext(nc) as tc:
        with (
            tc.tile_pool(name="sbuf", bufs=1) as sbuf,
            tc.tile_pool(name="psum", bufs=6, space="PSUM") as psum,
            tc.tile_pool(name="sc", bufs=2) as sc,
            tc.tile_pool(name="dram", bufs=1, space="DRAM") as dram,
        ):
            pools = (sbuf, psum, sc, dram)
            # ---- warmup collective first: its trigger starts the ~40us
            # CC-ring establishment clock, which runs while conv1 computes ----
            wu_s = sbuf.tile([1, 16], F32, name="wu_s")
            nc.vector.memset(wu_s[:], 0.0)
            wu_i = dram.tile([1, 16], F32, name="wu_i")
            wu_o = dram.tile([N_CORES, 16], F32, name="wu_o")
            nc.gpsimd.dma_start(wu_i[:], wu_s[:])
            nc.gpsimd.collective_compute(
                "AllGather", ALU.bypass,
                replica_groups=[list(range(N_CORES))],
                ins=[wu_i.opt()], outs=[wu_o.opt()])

            w1sb = sbuf.tile([128, 9 * NBLK * 256], FP8, name="w1sb")
            w2sb = sbuf.tile([128, 9 * NBLK * 256], FP8, name="w2sb")
            prm = sbuf.tile([128, NPRM], F32, name="prm")
            onesb = sbuf.tile([128, 256], FP8, name="onesb")
            nc.vector.memset(onesb[:], 1.0)
            xres = [sbuf.tile([128, BPC * PIX], F32, name=f"xres{i}")
                    for i in range(NBLK)]
            a1p = sbuf.tile([128, GUARD + BPC * SPLANE + GUARD], FP8,
                            name="a1p")
            a2p = sbuf.tile([128, GUARD + BPC * SPLANE + GUARD], FP8,
                            name="a2p")
            cv = [sbuf.tile([128, BPC * PIX], F32, name=f"cv{i}")
                  for i in range(NBLK)]
            xt = [sbuf.tile([128, BPC * PIX], F32, name=f"xt{i}")
                  for i in range(NBLK)]
            cv2 = [sbuf.tile([128, BPC * PIX], BF16, name=f"cv2{i}")
                   for i in range(NBLK)]
            xr2 = [sbuf.tile([128, BPC * PIX], BF16, name=f"xr2{i}")
                   for i in range(NBLK)]

            a1u = a1p[:].bitcast(mybir.dt.uint32)
            half_u = (GUARD + 2 * SPLANE) // 4
            nc.vector.memset(a1u[:, 0:half_u], 0)
            nc.vector.memset(a1u[:, half_u:], 0)
            xv = x_d[:].rearrange("b c h w -> c b (h w)")
            for b in range(BPC):
                for i in range(NBLK):
                    eng = nc.scalar if (b == 0 and i == 1) else nc.sync
                    eng.dma_start(
                        xres[i][:, b * PIX:(b + 1) * PIX],
                        xv[i * 128:(i + 1) * 128, b])
            # w1 split across three DMA queues so no single 8us transfer
            # gates the first matmul
            w1v = w1sb[:].rearrange("p (k i o) -> p k i o", k=9, i=NBLK)
            w1dv = w1_d[:].rearrange("k i p o -> p k i o")
            nc.scalar.dma_start(w1v[:, 0:3], w1dv[:, 0:3])
            nc.gpsimd.dma_start(w1v[:, 3:6], w1dv[:, 3:6])
            nc.sync.dma_start(w1v[:, 6:9], w1dv[:, 6:9])
            nc.sync.dma_start(prm[:], prm_d[:])
            nc.gpsimd.memset(a2p[:].bitcast(mybir.dt.uint32), 0)
            nc.sync.dma_start(
                w2sb[:].rearrange("p (k i o) -> p k i o", k=9, i=NBLK),
                w2_d[:].rearrange("k i p o -> p k i o"))

            # a1 = sign(x) into the padded interior, sample-outer layout so
            # sample b's conv work depends only on sample b's signs
            a1v = a1p[:, GUARD:GUARD + BPC * SPLANE].rearrange(
                "p (b i h w) -> p b i h w", b=BPC, i=2, h=HPAD, w=WPAD)
            xrvs = [xres[i][:].rearrange("p (b h w) -> p b h w", b=BPC, h=H)
                    for i in range(NBLK)]
            for b in range(BPC):
                for i in range(NBLK):
                    nc.scalar.activation(a1v[:, b, i, 1:1 + H, 1:1 + W],
                                         xrvs[i][:, b], AF.Sign)

            pcol1 = {'g': 0, 'b': 2, 'hp': 4, 'hs': 6, 'eps': 16, 'q': 20}
            pcol2 = {'g': 8, 'b': 10, 'hp': 12, 'hs': 14, 'eps': 18, 'q': 22}

            def mk_rhs(apad):
                t448 = apad[:, 0:SPLANE].rearrange(
                    "p (i n) -> p i n", i=2)[:, :, 0:RUN]

                def rhs_ap(off):
                    return bass.AP(t448.tensor, GUARD + off, t448.ap)
                return rhs_ap

            rhs1 = mk_rhs(a1p)
            rhs2 = mk_rhs(a2p)
            ones3 = onesb[:].rearrange("p (i o) -> p i o", i=2)
            wv1 = w1sb[:].rearrange("p (k i o) -> p k i o", k=9, i=NBLK)
            wv2 = w2sb[:].rearrange("p (k i o) -> p k i o", k=9, i=NBLK)

            a2iv = a2p[:, GUARD:GUARD + BPC * SPLANE].rearrange(
                "p (sp b2 i h w) -> p sp b2 i h w", sp=2, b2=2, i=2,
                h=HPAD, w=WPAD)
            ov = out_d[:].rearrange("(sp b2) c h w -> c sp b2 (h w)", sp=2)

            # ================= round 1: conv + stats =================
            sums1 = sbuf.tile([128, NCHUNK * NBLK], F32, name="sums1")
            sumsqs1 = sbuf.tile([128, NCHUNK * NBLK], F32, name="sumsqs1")
            s1s = {}
            for pair in range(2):
                for b in (2 * pair, 2 * pair + 1):
                    s1s[b] = _s1_sample(nc, sc, psum, 1, rhs1, ones3, b)
                for ci in range(4 * pair, 4 * pair + 4):
                    for oblk in range(NBLK):
                        _chunk_mms(nc, pools, 1, rhs1, wv1, s1s, cv, prm,
                                   pcol1, sums1, sumsqs1, ci, oblk)
            pair1 = sbuf.tile([128, NBLK * 2], F32, name="pair1")
            for oblk in range(NBLK):
                _local_pair(nc, sums1, sumsqs1, oblk, pair1, oblk * 2)
            g1_d = _ag_start(nc, dram, 1, pair1, NBLK, "")
            gst1 = _ag_gather(nc, sbuf, 1, g1_d, NBLK, "")
            sc1 = [_ag_finish(nc, sbuf, 1, gst1, prm, pcol1, oblk,
                              oblk * 2, f"{oblk}") for oblk in range(NBLK)]

            # ---- apply1 (light): t = scale*z + shift + x into xt;
            # a2 = sign(t).  The maxout multiply is deferred. ----
            for sp in range(2):
                for oblk in range(NBLK):
                    cvs = cv[oblk][:, sp * SPAN:(sp + 1) * SPAN]
                    xrs = xres[oblk][:, sp * SPAN:(sp + 1) * SPAN]
                    xts = xt[oblk][:, sp * SPAN:(sp + 1) * SPAN]
                    scale, shift = sc1[oblk]
                    nc.vector.scalar_tensor_tensor(
                        xts, cvs, scale[:], xrs, op0=ALU.mult, op1=ALU.add)
                    tv = xts.rearrange("p (b2 h w) -> p b2 h w", b2=2, h=H)
                    nc.scalar.activation(a2iv[:, sp, :, oblk, 1:1 + H,
                                              1:1 + W], tv, AF.Sign,
                                         bias=shift[:], scale=1.0)

            # ================= round 2: conv (oblk-major) =================
            sums2 = sbuf.tile([128, NCHUNK * NBLK], F32, name="sums2")
            sumsqs2 = sbuf.tile([128, NCHUNK * NBLK], F32, name="sumsqs2")

            s2s = {}
            for pair in range(2):
                for b in (2 * pair, 2 * pair + 1):
                    s2s[b] = _s1_sample(nc, sc, psum, 2, rhs2, ones3, b)
                for ci in range(4 * pair, 4 * pair + 4):
                    _chunk_mms(nc, pools, 2, rhs2, wv2, s2s, cv2, prm,
                               pcol2, sums2, sumsqs2, ci, 0)
            pair2 = sbuf.tile([128, NBLK * 2], F32, name="pair2")
            _local_pair(nc, sums2, sumsqs2, 0, pair2, 0)

            # x1 = t * (hp*sign(t) + hs): deferred maxout, off the AG2a
            # critical path, overlapped with conv2's oblk-1 sweep
            for sp in range(2):
                for oblk in range(NBLK):
                    xts = xt[oblk][:, sp * SPAN:(sp + 1) * SPAN]
                    xrs = xres[oblk][:, sp * SPAN:(sp + 1) * SPAN]
                    coef = sc.tile([128, SPAN], F32, tag="coef", name="coef",
                                   bufs=2)
                    nc.scalar.activation(
                        coef[:].rearrange("p (b2 h w) -> p b2 h w",
                                          b2=2, h=H),
                        a2iv[:, sp, :, oblk, 1:1 + H, 1:1 + W], AF.Identity,
                        bias=prm[:, pcol1['hs'] + oblk:
                                 pcol1['hs'] + oblk + 1],
                        scale=prm[:, pcol1['hp'] + oblk:
                                  pcol1['hp'] + oblk + 1])
                    nc.vector.scalar_tensor_tensor(
                        xr2[oblk][:, sp * SPAN:(sp + 1) * SPAN], xts,
                        sc1[oblk][1][:], coef[:],
                        op0=ALU.add, op1=ALU.mult)

            for ci in range(NCHUNK):
                _chunk_mms(nc, pools, 2, rhs2, wv2, s2s, cv2, prm,
                           pcol2, sums2, sumsqs2, ci, 1)
            _local_pair(nc, sums2, sumsqs2, 1, pair2, 2)
            g2_d = _ag_start(nc, dram, 2, pair2, NBLK, "")
            gst2 = _ag_gather(nc, sbuf, 2, g2_d, NBLK, "")

            # ---- apply2: one AG for both oblks, bf16 elementwise ----
            sc2 = [_ag_finish(nc, sbuf, 2, gst2, prm, pcol2, oblk,
                              oblk * 2, "ab"[oblk]) for oblk in range(NBLK)]
            for oblk in range(NBLK):
                scale, shift = sc2[oblk]
                for sp in range(2):
                    cvs = cv2[oblk][:, sp * SPAN:(sp + 1) * SPAN]
                    xrs = xr2[oblk][:, sp * SPAN:(sp + 1) * SPAN]
                    t2 = sc.tile([128, SPAN], BF16, tag="u2", name="u2",
                                 bufs=4)
                    nc.vector.scalar_tensor_tensor(
                        t2[:], cvs, scale[:], xrs,
                        op0=ALU.mult, op1=ALU.add)
                    sgt = sc.tile([128, SPAN], BF16, tag="sg", name="sg",
                                  bufs=2)
                    nc.scalar.activation(sgt[:], t2[:], AF.Sign,
                                         bias=shift[:], scale=1.0)
                    coef = sc.tile([128, SPAN], BF16, tag="coef2",
                                   name="coef2", bufs=2)
                    nc.vector.tensor_scalar(
                        coef[:], sgt[:],
                        prm[:, pcol2['hp'] + oblk:pcol2['hp'] + oblk + 1],
                        prm[:, pcol2['hs'] + oblk:pcol2['hs'] + oblk + 1],
                        op0=ALU.mult, op1=ALU.add)
                    t2h = t2[:].rearrange("p (b2 hw) -> p b2 hw", b2=2)
                    cfh = coef[:].rearrange("p (b2 hw) -> p b2 hw", b2=2)
                    for b2 in range(2):
                        och = sc.tile([128, PIX], F32, tag="och",
                                      name="och", bufs=3)
                        nc.vector.scalar_tensor_tensor(
                            och[:], t2h[:, b2], shift[:], cfh[:, b2],
                            op0=ALU.add, op1=ALU.mult)
                        nc.sync.dma_start(
                            ov[oblk * 128:oblk * 128 + 128, sp, b2],
                            och[:])

    nc.compile()
    return nc


def _prep_weight(w):
    """(O,I,3,3) fp32 -> sign lhsT (9, iblk, 128, 256) fp8 (+-1, exact),
    plus per-output-channel alpha, beta (float64)."""
    w = w.astype(np.float64)
    beta = w.mean(axis=(1, 2, 3))
    alpha = np.sqrt(((w - beta[:, None, None, None]) ** 2)
                    .mean(axis=(1, 2, 3)))
    s = np.sign(w - beta[:, None, None, None]).astype(np.float32)
    wt = s.transpose(2, 3, 1, 0).reshape(9, C, C)   # (k9, i, o)
    wt = wt.reshape(9, NBLK, 128, C)                # (k9, iblk, i, o)
    return wt.astype(ml_dtypes.float8_e4m3), alpha, beta


def make_in_maps(inputs):
    x = np.asarray(inputs['x'], np.float32)
    aa1 = float(np.asarray(inputs['alpha_a1']).reshape(-1)[0])
    aa2 = float(np.asarray(inputs['alpha_a2']).reshape(-1)[0])
    w1t, al1, be1 = _prep_weight(np.asarray(inputs['w1'], np.float32))
    w2t, al2, be2 = _prep_weight(np.asarray(inputs['w2'], np.float32))
    prm = np.zeros((128, NPRM), np.float32)
    f1 = 1.0 / (aa1 * al1)      # z scale relative to the true conv output
    f2 = 1.0 / (aa2 * al2)
    p1 = np.asarray(inputs['pos1'], np.float64)
    n1 = np.asarray(inputs['neg1'], np.float64)
    p2 = np.asarray(inputs['pos2'], np.float64)
    n2 = np.asarray(inputs['neg2'], np.float64)
    cols = ((0, np.asarray(inputs['g1'], np.float64)),
            (2, np.asarray(inputs['b1'], np.float64)),
            (4, (p1 - n1) / 2),
            (6, (p1 + n1) / 2),
            (8, np.asarray(inputs['g2'], np.float64)),
            (10, np.asarray(inputs['b2'], np.float64)),
            (12, (p2 - n2) / 2),
            (14, (p2 + n2) / 2),
            (16, BN_EPS * f1 * f1),
            (18, BN_EPS * f2 * f2),
            (20, be1 / al1),
            (22, be2 / al2))
    for base, arr in cols:
        prm[:, base] = arr[:128]
        prm[:, base + 1] = arr[128:]
    in_maps = []
    for c in range(N_CORES):
        in_maps.append({
            'x': np.ascontiguousarray(x[c * BPC:(c + 1) * BPC]),
            'w1t': w1t, 'w2t': w2t, 'prm': prm,
        })
    return in_maps


_CACHE = {}


def kernel(**inputs):
    in_maps = make_in_maps(inputs)
    if 'run' not in _CACHE:
        nc = build()
        _CACHE['nc'] = nc
        _CACHE['run'] = _make_runner(nc)
    outs = _CACHE['run'](in_maps)
    return np.concatenate([outs[c] for c in range(N_CORES)], axis=0)


def _make_runner(nc):
    """Build a cached PJRT executable (same path run_bass_kernel_spmd takes
    under axon, via bass2jax) so repeat calls don't re-trace."""
    import jax
    import jax.numpy as jnp
    from jax.sharding import Mesh, PartitionSpec
    from jax.experimental.shard_map import shard_map
    from concourse import bass2jax

    bass2jax.install_neuronx_cc_hook()
    partition_name = (nc.partition_id_tensor.name
                      if nc.partition_id_tensor else None)
    in_names = []
    out_names = []
    out_avals = []
    for alloc in nc.m.functions[0].allocations:
        if not isinstance(alloc, mybir.MemoryLocationSet):
            continue
        name = alloc.memorylocations[0].name
        if alloc.kind == "ExternalInput":
            if name != partition_name:
                in_names.append(name)
        elif alloc.kind == "ExternalOutput":
            shape = tuple(alloc.tensor_shape)
            dtype = mybir.dt.np(alloc.dtype)
            out_names.append(name)
            out_avals.append(jax.core.ShapedArray(shape, dtype))
    n_params = len(in_names)
    all_names = in_names + out_names
    if partition_name is not None:
        all_names = all_names + [partition_name]

    def _body(*args):
        operands = list(args)
        if partition_name is not None:
            operands.append(bass2jax.partition_id_tensor())
        outs = bass2jax._bass_exec_p.bind(
            *operands,
            out_avals=tuple(out_avals),
            in_names=tuple(all_names),
            out_names=tuple(out_names),
            lowering_input_output_aliases=(),
            sim_require_finite=True,
            sim_require_nnan=True,
            nc=nc,
        )
        return tuple(outs)

    devices = jax.devices()[:N_CORES]
    mesh = Mesh(np.asarray(devices), ("core",))
    n_outs = len(out_names)
    sharded = jax.jit(
        shard_map(_body, mesh=mesh,
                  in_specs=(PartitionSpec("core"),) * (n_params + n_outs),
                  out_specs=(PartitionSpec("core"),) * n_outs,
                  check_rep=False),
        donate_argnums=tuple(range(n_params, n_params + n_outs)),
        keep_unused=True,
    )
    sharded_nodonate = jax.jit(
        shard_map(_body, mesh=mesh,
                  in_specs=(PartitionSpec("core"),) * (n_params + n_outs),
                  out_specs=(PartitionSpec("core"),) * n_outs,
                  check_rep=False),
        keep_unused=True,
    )

    def run(in_maps):
        concat_in = [
            np.concatenate([np.asarray(in_maps[c][n]) for c in range(N_CORES)],
                           axis=0)
            for n in in_names
        ]
        concat_zeros = [
            np.zeros((N_CORES * a.shape[0], *a.shape[1:]), a.dtype)
            for a in out_avals
        ]
        out_arrs = sharded(*concat_in, *concat_zeros)
        i = out_names.index("out")
        full = np.asarray(out_arrs[i]).reshape(N_CORES, *out_avals[i].shape)
        return [full[c] for c in range(N_CORES)]

    def stage(in_maps):
        """device_put inputs once; return a dispatch closure for timing."""
        from jax.sharding import NamedSharding
        sh = NamedSharding(mesh, PartitionSpec("core"))
        concat_in = [
            jax.device_put(np.concatenate(
                [np.asarray(in_maps[c][n]) for c in range(N_CORES)], axis=0), sh)
            for n in in_names
        ]
        concat_zeros = [
            jax.device_put(
                np.zeros((N_CORES * a.shape[0], *a.shape[1:]), a.dtype), sh)
            for a in out_avals
        ]

        def dispatch():
            return sharded_nodonate(*concat_in, *concat_zeros)

        return dispatch

    run.stage = stage
    return run


# revision 32
# speedup vs baseline: 1.0329x; 1.0329x over previous
"""Trainium2 Bass kernel for a binarized (1w/1a) BasicBlock — fp8 DoubleRow.

    a1 = sign(x);  y1 = BN(conv3x3(a1, binarize(w1))) + x;  x1 = maxout(y1)
    a2 = sign(x1); y2 = BN(conv3x3(a2, binarize(w2))) + x1; out = maxout(y2)

Data-parallel over batch (4 samples/core, 8 cores); exact binary math:
activations are +-1 (fp8e4, exact), weights are sign(+-1) fp8; each conv is
9 DoubleRow matmuls per (chunk, cout-block), contracting all 256 input
channels at once over contiguous padded-row runs (pad columns land in
unused psum columns).  conv_true = alpha_a*alpha[o]*(BB + q[o]*S1) with
q = beta/alpha; S1 (3x3 box of the channel sum) comes from 3 more DoubleRow
ones-matmuls (folding the kh taps) + 2 shifted adds.  The per-channel scale
folds into BN exactly by scaling BN_EPS per channel.  Batch-stat BN uses an
AllGather of per-core (count, mean, M2) triplets + bn_aggr on every core.

Scheduling (v2): sample-outer padded activation layout so the conv stream
starts as soon as the first sample pair is signed; one early warmup
AllGather absorbs the CC-ring establishment; apply1 only computes
t = BN(z)+x (into the dead cv space) and sign(t) -> a2, deferring the
maxout multiply (x1 = t*coef) into the conv2 window; round-2 conv runs
cout-block-major with a per-block stats AllGather so the second collective
overlaps conv2 and apply2(block0).
"""

import numpy as np
import ml_dtypes

import concourse.bass as bass
import concourse.bacc as bacc
import concourse.mybir as mybir
import concourse.tile as tile

N_CORES = 8
B, C, H, W = 32, 256, 28, 28
BPC = B // N_CORES            # samples per core
NBLK = 2                      # channel blocks of 128
HPAD, WPAD = 30, 30           # padded image in SBUF
PIX = H * W                   # 784
PPIX = HPAD * WPAD            # 900
SPLANE = 2 * PPIX             # both channel planes of one sample: 1800
NCHUNK = 2 * BPC              # 8 chunks of (sample, half-image)
HHALF = H // 2                # 14
CHUNK = HHALF * W             # 392 dense output elems per chunk
RUN = HHALF * WPAD            # 420: rhs run length / psum width per chunk
BN_EPS = 1e-5
NPRM = 24
GUARD = 32                    # fp8 guard elems around merged activation tile
SPAN = 4 * CHUNK              # 1568-wide apply spans (2 samples)
F32 = mybir.dt.float32
BF16 = mybir.dt.bfloat16
FP8 = mybir.dt.float8e4
AF = mybir.ActivationFunctionType
ALU = mybir.AluOpType
DR = mybir.MatmulPerfMode.DoubleRow


def _evac(nc, sc, ps, s1, sums, sumsqs, cv, prm, pcol, ci, oblk):
    """z = q[o]*S1 + BB from PSUM (strided: skip pad cols).  Stats come for
    free: the STT accumulates sum(z) on DVE; a Square pass on the otherwise
    idle ScalarE accumulates sum(z^2)."""
    psv = ps[:].rearrange("p (h w) -> p h w", h=HHALF)[:, :, 1:1 + W]
    s1v = s1[:].rearrange("p (h w) -> p h w", h=H)[
        :, (ci % 2) * HHALF:(ci % 2) * HHALF + HHALF, :]
    cvc = cv[oblk][:, ci * CHUNK:(ci + 1) * CHUNK]
    nc.vector.scalar_tensor_tensor(
        cvc.rearrange("p (h w) -> p h w", h=HHALF), s1v,
        prm[:, pcol['q'] + oblk:pcol['q'] + oblk + 1], psv,
        op0=ALU.mult, op1=ALU.add,
        accum_out=sums[:, oblk * NCHUNK + ci:oblk * NCHUNK + ci + 1])
    sqj = sc.tile([128, CHUNK], F32, tag="sqj", name="sqj", bufs=2)
    nc.scalar.activation(
        sqj[:], cvc, AF.Square,
        accum_out=sumsqs[:, oblk * NCHUNK + ci:oblk * NCHUNK + ci + 1])


def _s1_sample(nc, sc, psum, rnd, rhs_ap, ones3, b):
    """S1 (3x3 box of channel sums) for one sample: 3 kh-folding DoubleRow
    ones-matmuls per half + 2 shifted W-direction adds."""
    hs = sc.tile([128, 2 * RUN], F32, tag="hs", name="hs", bufs=2)
    for half in range(2):
        h0 = half * HHALF
        ps2 = psum.tile([128, RUN], F32, tag="ps2", name=f"ps2_{rnd}",
                        bufs=2)
        for kh in range(3):
            nc.tensor.matmul(ps2[:], ones3,
                             rhs_ap(b * SPLANE + (h0 + kh) * WPAD),
                             start=(kh == 0), stop=(kh == 2), perf_mode=DR)
        nc.scalar.copy(hs[:, half * RUN:half * RUN + RUN], ps2[:])
    hsv = hs[:].rearrange("p (h w) -> p h w", h=H)
    w3 = sc.tile([128, H * W], F32, tag="w3", name="w3", bufs=2)
    w3v = w3[:].rearrange("p (h w) -> p h w", h=H)
    nc.vector.tensor_add(w3v, hsv[:, :, 0:W], hsv[:, :, 1:1 + W])
    s1 = sc.tile([128, H * W], F32, tag="s1", name="s1", bufs=BPC)
    s1v = s1[:].rearrange("p (h w) -> p h w", h=H)
    nc.vector.tensor_add(s1v, w3v, hsv[:, :, 2:2 + W])
    return s1


def _chunk_mms(nc, pools, rnd, rhs_ap, wv, s1s, cv, prm, pcol, sums, sumsqs,
               ci, oblk):
    sbuf, psum, sc, dram = pools
    b, h0 = ci // 2, (ci % 2) * HHALF
    ps = psum.tile([128, RUN], F32, tag="ps", name=f"ps{rnd}", bufs=6)
    for k9 in range(9):
        kh, kw = k9 // 3, k9 % 3
        nc.tensor.matmul(
            ps[:], wv[:, k9, :, oblk * 128:(oblk + 1) * 128],
            rhs_ap(b * SPLANE + (h0 + kh) * WPAD + kw - 1),
            start=(k9 == 0), stop=(k9 == 8), perf_mode=DR)
    _evac(nc, sc, ps, s1s[b], sums, sumsqs, cv, prm, pcol, ci, oblk)


def _local_pair(nc, sums, sumsqs, oblk, dest, col):
    """Raw (sum, sumsq) over this oblk's 8 chunks -> dest[:, col:col+2].
    Only two reduces sit before the AllGather trigger."""
    nc.vector.reduce_sum(dest[:, col:col + 1],
                         sums[:, oblk * NCHUNK:(oblk + 1) * NCHUNK],
                         axis=mybir.AxisListType.X)
    nc.vector.reduce_sum(dest[:, col + 1:col + 2],
                         sumsqs[:, oblk * NCHUNK:(oblk + 1) * NCHUNK],
                         axis=mybir.AxisListType.X)


def _ag_start(nc, dram, rnd, pair, npair, tag):
    """DMA the local raw sums to DRAM and trigger the AllGather."""
    b_d = dram.tile([128, 2 * npair], F32, name=f"bd{rnd}{tag}")
    g_d = dram.tile([N_CORES, 128, 2 * npair], F32, name=f"gd{rnd}{tag}")
    nc.gpsimd.dma_start(b_d[:], pair[:])
    nc.gpsimd.collective_compute(
        "AllGather", ALU.bypass,
        replica_groups=[list(range(N_CORES))],
        ins=[b_d.opt()], outs=[g_d.opt()])
    return g_d


def _ag_gather(nc, sbuf, rnd, g_d, npair, tag):
    """Gather back r-major (contiguous per-rank j-tuples -> one fast DMA);
    the per-stat reduction later uses a strided AP."""
    gst = sbuf.tile([128, 2 * npair * N_CORES], F32, name=f"gst{rnd}{tag}")
    nc.sync.dma_start(
        gst[:].rearrange("p (r j) -> p r j", r=N_CORES),
        g_d[:].rearrange("r p j -> p r j"))
    return gst


def _ag_finish(nc, sbuf, rnd, gst, prm, pcol, oblk, jcol, tag):
    """Global mean/var from raw sums; per-channel scale/shift columns."""
    cnt = float(N_CORES * NCHUNK * CHUNK)
    mean = sbuf.tile([128, 1], F32, name=f"mean{rnd}{tag}")
    ms = sbuf.tile([128, 1], F32, name=f"ms{rnd}{tag}")
    m2 = sbuf.tile([128, 1], F32, name=f"m2{rnd}{tag}")
    var = sbuf.tile([128, 1], F32, name=f"var{rnd}{tag}")
    sd = sbuf.tile([128, 1], F32, name=f"sd{rnd}{tag}")
    inv = sbuf.tile([128, 1], F32, name=f"inv{rnd}{tag}")
    scale = sbuf.tile([128, 1], F32, name=f"scale{rnd}{tag}")
    tmp = sbuf.tile([128, 1], F32, name=f"tmp{rnd}{tag}")
    shift = sbuf.tile([128, 1], F32, name=f"shift{rnd}{tag}")
    gv = gst[:].rearrange("p (r j) -> p j r", r=N_CORES)
    nc.vector.reduce_sum(tmp[:], gv[:, jcol], axis=mybir.AxisListType.X)
    nc.vector.tensor_scalar_mul(mean[:], tmp[:], 1.0 / cnt)
    nc.vector.reduce_sum(tmp[:], gv[:, jcol + 1], axis=mybir.AxisListType.X)
    nc.vector.tensor_scalar_mul(ms[:], tmp[:], 1.0 / cnt)
    nc.vector.tensor_mul(m2[:], mean[:], mean[:])
    nc.vector.tensor_sub(var[:], ms[:], m2[:])
    nc.scalar.activation(sd[:], var[:], AF.Sqrt,
                         bias=prm[:, pcol['eps'] + oblk:
                                  pcol['eps'] + oblk + 1],
                         scale=1.0)
    nc.vector.reciprocal(inv[:], sd[:])
    nc.vector.tensor_mul(scale[:], inv[:],
                         prm[:, pcol['g'] + oblk:pcol['g'] + oblk + 1])
    nc.vector.tensor_mul(tmp[:], mean[:], scale[:])
    nc.vector.tensor_sub(shift[:],
                         prm[:, pcol['b'] + oblk:pcol['b'] + oblk + 1],
                         tmp[:])
    return scale, shift


def build():
    nc = bacc.Bacc("TRN2", target_bir_lowering=False, debug=False,
                   enable_asserts=True, num_devices=N_CORES)
    x_d = nc.dram_tensor("x", [BPC, C, H, W], F32, kind="ExternalInput")
    w1_d = nc.dram_tensor("w1t", [9, NBLK, 128, 256], FP8,
                          kind="ExternalInput")
    w2_d = nc.dram_tensor("w2t", [9, NBLK, 128, 256], FP8,
                          kind="ExternalInput")
    prm_d = nc.dram_tensor("prm", [128, NPRM], F32, kind="ExternalInput")
    out_d = nc.dram_tensor("out", [BPC, C, H, W], F32, kind="ExternalOutput")

    with tile.TileContext(nc) as tc:
        with (
            tc.tile_pool(name="sbuf", bufs=1) as sbuf,
            tc.tile_pool(name="psum", bufs=6, space="PSUM") as psum,
            tc.tile_pool(name="sc", bufs=2) as sc,
            tc.tile_pool(name="dram", bufs=1, space="DRAM") as dram,
        ):
            pools = (sbuf, psum, sc, dram)
            # ---- warmup collective first: its trigger starts the ~40us
            # CC-ring establishment clock, which runs while conv1 computes ----
            wu_s = sbuf.tile([1, 16], F32, name="wu_s")
            nc.vector.memset(wu_s[:], 0.0)
            wu_i = dram.tile([1, 16], F32, name="wu_i")
            wu_o = dram.tile([N_CORES, 16], F32, name="wu_o")
            nc.gpsimd.dma_start(wu_i[:], wu_s[:])
            nc.gpsimd.collective_compute(
                "AllGather", ALU.bypass,
                replica_groups=[list(range(N_CORES))],
                ins=[wu_i.opt()], outs=[wu_o.opt()])

            w1sb = sbuf.tile([128, 9 * NBLK * 256], FP8, name="w1sb")
            w2sb = sbuf.tile([128, 9 * NBLK * 256], FP8, name="w2sb")
            prm = sbuf.tile([128, NPRM], F32, name="prm")
            onesb = sbuf.tile([128, 256], FP8, name="onesb")
            nc.vector.memset(onesb[:], 1.0)
            xres = [sbuf.tile([128, BPC * PIX], F32, name=f"xres{i}")
                    for i in range(NBLK)]
            a1p = sbuf.tile([128, GUARD + BPC * SPLANE + GUARD], FP8,
                            name="a1p")
            a2p = sbuf.tile([128, GUARD + BPC * SPLANE + GUARD], FP8,
                            name="a2p")
            cv = [sbuf.tile([128, BPC * PIX], F32, name=f"cv{i}")
                  for i in range(NBLK)]
            xt = [sbuf.tile([128, BPC * PIX], F32, name=f"xt{i}")
                  for i in range(NBLK)]
            cv2 = [sbuf.tile([128, BPC * PIX], BF16, name=f"cv2{i}")
                   for i in range(NBLK)]
            xr2 = [sbuf.tile([128, BPC * PIX], BF16, name=f"xr2{i}")
                   for i in range(NBLK)]

            a1u = a1p[:].bitcast(mybir.dt.uint32)
            half_u = (GUARD + 2 * SPLANE) // 4
            nc.vector.memset(a1u[:, 0:half_u], 0)
            nc.vector.memset(a1u[:, half_u:], 0)
            xv = x_d[:].rearrange("b c h w -> c b (h w)")
            for b in range(BPC):
                for i in range(NBLK):
                    nc.sync.dma_start(
                        xres[i][:, b * PIX:(b + 1) * PIX],
                        xv[i * 128:(i + 1) * 128, b])
            # w1 split across three DMA queues so no single 8us transfer
            # gates the first matmul
            w1v = w1sb[:].rearrange("p (k i o) -> p k i o", k=9, i=NBLK)
            w1dv = w1_d[:].rearrange("k i p o -> p k i o")
            nc.scalar.dma_start(w1v[:, 0:3], w1dv[:, 0:3])
            nc.gpsimd.dma_start(w1v[:, 3:6], w1dv[:, 3:6])
            nc.sync.dma_start(w1v[:, 6:9], w1dv[:, 6:9])
            nc.sync.dma_start(prm[:], prm_d[:])
            nc.gpsimd.memset(a2p[:].bitcast(mybir.dt.uint32), 0)
            nc.sync.dma_start(
                w2sb[:].rearrange("p (k i o) -> p k i o", k=9, i=NBLK),
                w2_d[:].rearrange("k i p o -> p k i o"))

            # a1 = sign(x) into the padded interior, sample-outer layout so
            # sample b's conv work depends only on sample b's signs
            a1v = a1p[:, GUARD:GUARD + BPC * SPLANE].rearrange(
                "p (b i h w) -> p b i h w", b=BPC, i=2, h=HPAD, w=WPAD)
            xrvs = [xres[i][:].rearrange("p (b h w) -> p b h w", b=BPC, h=H)
                    for i in range(NBLK)]
            for b in range(BPC):
                for i in range(NBLK):
                    nc.scalar.activation(a1v[:, b, i, 1:1 + H, 1:1 + W],
                                         xrvs[i][:, b], AF.Sign)

            pcol1 = {'g': 0, 'b': 2, 'hp': 4, 'hs': 6, 'eps': 16, 'q': 20}
            pcol2 = {'g': 8, 'b': 10, 'hp': 12, 'hs': 14, 'eps': 18, 'q': 22}

            def mk_rhs(apad):
                t448 = apad[:, 0:SPLANE].rearrange(
                    "p (i n) -> p i n", i=2)[:, :, 0:RUN]

                def rhs_ap(off):
                    return bass.AP(t448.tensor, GUARD + off, t448.ap)
                return rhs_ap

            rhs1 = mk_rhs(a1p)
            rhs2 = mk_rhs(a2p)
            ones3 = onesb[:].rearrange("p (i o) -> p i o", i=2)
            wv1 = w1sb[:].rearrange("p (k i o) -> p k i o", k=9, i=NBLK)
            wv2 = w2sb[:].rearrange("p (k i o) -> p k i o", k=9, i=NBLK)

            a2iv = a2p[:, GUARD:GUARD + BPC * SPLANE].rearrange(
                "p (sp b2 i h w) -> p sp b2 i h w", sp=2, b2=2, i=2,
                h=HPAD, w=WPAD)
            ov = out_d[:].rearrange("(sp b2) c h w -> c sp b2 (h w)", sp=2)

            # ================= round 1: conv + stats =================
            sums1 = sbuf.tile([128, NCHUNK * NBLK], F32, name="sums1")
            sumsqs1 = sbuf.tile([128, NCHUNK * NBLK], F32, name="sumsqs1")
            s1s = {}
            for pair in range(2):
                for b in (2 * pair, 2 * pair + 1):
                    s1s[b] = _s1_sample(nc, sc, psum, 1, rhs1, ones3, b)
                for ci in range(4 * pair, 4 * pair + 4):
                    for oblk in range(NBLK):
                        _chunk_mms(nc, pools, 1, rhs1, wv1, s1s, cv, prm,
                                   pcol1, sums1, sumsqs1, ci, oblk)
            pair1 = sbuf.tile([128, NBLK * 2], F32, name="pair1")
            for oblk in range(NBLK):
                _local_pair(nc, sums1, sumsqs1, oblk, pair1, oblk * 2)
            g1_d = _ag_start(nc, dram, 1, pair1, NBLK, "")
            gst1 = _ag_gather(nc, sbuf, 1, g1_d, NBLK, "")
            sc1 = [_ag_finish(nc, sbuf, 1, gst1, prm, pcol1, oblk,
                              oblk * 2, f"{oblk}") for oblk in range(NBLK)]

            # ---- apply1 (light): t = scale*z + shift + x into xt;
            # a2 = sign(t).  The maxout multiply is deferred. ----
            for sp in range(2):
                for oblk in range(NBLK):
                    cvs = cv[oblk][:, sp * SPAN:(sp + 1) * SPAN]
                    xrs = xres[oblk][:, sp * SPAN:(sp + 1) * SPAN]
                    xts = xt[oblk][:, sp * SPAN:(sp + 1) * SPAN]
                    scale, shift = sc1[oblk]
                    nc.vector.scalar_tensor_tensor(
                        xts, cvs, scale[:], xrs, op0=ALU.mult, op1=ALU.add)
                    tv = xts.rearrange("p (b2 h w) -> p b2 h w", b2=2, h=H)
                    nc.scalar.activation(a2iv[:, sp, :, oblk, 1:1 + H,
                                              1:1 + W], tv, AF.Sign,
                                         bias=shift[:], scale=1.0)

            # ================= round 2: conv (oblk-major) =================
            sums2 = sbuf.tile([128, NCHUNK * NBLK], F32, name="sums2")
            sumsqs2 = sbuf.tile([128, NCHUNK * NBLK], F32, name="sumsqs2")

            s2s = {}
            for pair in range(2):
                for b in (2 * pair, 2 * pair + 1):
                    s2s[b] = _s1_sample(nc, sc, psum, 2, rhs2, ones3, b)
                for ci in range(4 * pair, 4 * pair + 4):
                    _chunk_mms(nc, pools, 2, rhs2, wv2, s2s, cv2, prm,
                               pcol2, sums2, sumsqs2, ci, 0)
            pair2 = sbuf.tile([128, NBLK * 2], F32, name="pair2")
            _local_pair(nc, sums2, sumsqs2, 0, pair2, 0)

            # x1 = t * (hp*sign(t) + hs): deferred maxout, off the AG2a
            # critical path, overlapped with conv2's oblk-1 sweep
            for sp in range(2):
                for oblk in range(NBLK):
                    xts = xt[oblk][:, sp * SPAN:(sp + 1) * SPAN]
                    xrs = xres[oblk][:, sp * SPAN:(sp + 1) * SPAN]
                    coef = sc.tile([128, SPAN], F32, tag="coef", name="coef",
                                   bufs=2)
                    nc.scalar.activation(
                        coef[:].rearrange("p (b2 h w) -> p b2 h w",
                                          b2=2, h=H),
                        a2iv[:, sp, :, oblk, 1:1 + H, 1:1 + W], AF.Identity,
                        bias=prm[:, pcol1['hs'] + oblk:
                                 pcol1['hs'] + oblk + 1],
                        scale=prm[:, pcol1['hp'] + oblk:
                                  pcol1['hp'] + oblk + 1])
                    nc.vector.scalar_tensor_tensor(
                        xr2[oblk][:, sp * SPAN:(sp + 1) * SPAN], xts,
                        sc1[oblk][1][:], coef[:],
                        op0=ALU.add, op1=ALU.mult)

            for ci in range(NCHUNK):
                _chunk_mms(nc, pools, 2, rhs2, wv2, s2s, cv2, prm,
                           pcol2, sums2, sumsqs2, ci, 1)
            _local_pair(nc, sums2, sumsqs2, 1, pair2, 2)
            g2_d = _ag_start(nc, dram, 2, pair2, NBLK, "")
            gst2 = _ag_gather(nc, sbuf, 2, g2_d, NBLK, "")

            # ---- apply2: one AG for both oblks, bf16 elementwise ----
            sc2 = [_ag_finish(nc, sbuf, 2, gst2, prm, pcol2, oblk,
                              oblk * 2, "ab"[oblk]) for oblk in range(NBLK)]
            for oblk in range(NBLK):
                scale, shift = sc2[oblk]
                for sp in range(2):
                    cvs = cv2[oblk][:, sp * SPAN:(sp + 1) * SPAN]
                    xrs = xr2[oblk][:, sp * SPAN:(sp + 1) * SPAN]
                    t2 = sc.tile([128, SPAN], BF16, tag="u2", name="u2",
                                 bufs=4)
                    nc.vector.scalar_tensor_tensor(
                        t2[:], cvs, scale[:], xrs,
                        op0=ALU.mult, op1=ALU.add)
                    sgt = sc.tile([128, SPAN], BF16, tag="sg", name="sg",
                                  bufs=2)
                    nc.scalar.activation(sgt[:], t2[:], AF.Sign,
                                         bias=shift[:], scale=1.0)
                    coef = sc.tile([128, SPAN], BF16, tag="coef2",
                                   name="coef2", bufs=2)
                    nc.vector.tensor_scalar(
                        coef[:], sgt[:],
                        prm[:, pcol2['hp'] + oblk:pcol2['hp'] + oblk + 1],
                        prm[:, pcol2['hs'] + oblk:pcol2['hs'] + oblk + 1],
                        op0=ALU.mult, op1=ALU.add)
                    t2h = t2[:].rearrange("p (b2 hw) -> p b2 hw", b2=2)
                    cfh = coef[:].rearrange("p (b2 hw) -> p b2 hw", b2=2)
                    for b2 in range(2):
                        och = sc.tile([128, PIX], F32, tag="och",
                                      name="och", bufs=3)
                        nc.vector.scalar_tensor_tensor(
                            och[:], t2h[:, b2], shift[:], cfh[:, b2],
                            op0=ALU.add, op1=ALU.mult)
                        nc.sync.dma_start(
                            ov[oblk * 128:oblk * 128 + 128, sp, b2],
                            och[:])

    nc.compile()
    return nc


def _prep_weight(w):
    """(O,I,3,3) fp32 -> sign lhsT (9, iblk, 128, 256) fp8 (+-1, exact),
    plus per-output-channel alpha, beta (float64)."""
    w = w.astype(np.float64)
    beta = w.mean(axis=(1, 2, 3))
    alpha = np.sqrt(((w - beta[:, None, None, None]) ** 2)
                    .mean(axis=(1, 2, 3)))
    s = np.sign(w - beta[:, None, None, None]).astype(np.float32)
    wt = s.transpose(2, 3, 1, 0).reshape(9, C, C)   # (k9, i, o)
    wt = wt.reshape(9, NBLK, 128, C)                # (k9, iblk, i, o)
    return wt.astype(ml_dtypes.float8_e4m3), alpha, beta


def make_in_maps(inputs):
    x = np.asarray(inputs['x'], np.float32)
    aa1 = float(np.asarray(inputs['alpha_a1']).reshape(-1)[0])
    aa2 = float(np.asarray(inputs['alpha_a2']).reshape(-1)[0])
    w1t, al1, be1 = _prep_weight(np.asarray(inputs['w1'], np.float32))
    w2t, al2, be2 = _prep_weight(np.asarray(inputs['w2'], np.float32))
    prm = np.zeros((128, NPRM), np.float32)
    f1 = 1.0 / (aa1 * al1)      # z scale relative to the true conv output
    f2 = 1.0 / (aa2 * al2)
    p1 = np.asarray(inputs['pos1'], np.float64)
    n1 = np.asarray(inputs['neg1'], np.float64)
    p2 = np.asarray(inputs['pos2'], np.float64)
    n2 = np.asarray(inputs['neg2'], np.float64)
    cols = ((0, np.asarray(inputs['g1'], np.float64)),
            (2, np.asarray(inputs['b1'], np.float64)),
            (4, (p1 - n1) / 2),
            (6, (p1 + n1) / 2),
            (8, np.asarray(inputs['g2'], np.float64)),
            (10, np.asarray(inputs['b2'], np.float64)),
            (12, (p2 - n2) / 2),
            (14, (p2 + n2) / 2),
            (16, BN_EPS * f1 * f1),
            (18, BN_EPS * f2 * f2),
            (20, be1 / al1),
            (22, be2 / al2))
    for base, arr in cols:
        prm[:, base] = arr[:128]
        prm[:, base + 1] = arr[128:]
    in_maps = []
    for c in range(N_CORES):
        in_maps.append({
            'x': np.ascontiguousarray(x[c * BPC:(c + 1) * BPC]),
            'w1t': w1t, 'w2t': w2t, 'prm': prm,
        })
    return in_maps


_CACHE = {}


def kernel(**inputs):
    in_maps = make_in_maps(inputs)
    if 'run' not in _CACHE:
        nc = build()
        _CACHE['nc'] = nc
        _CACHE['run'] = _make_runner(nc)
    outs = _CACHE['run'](in_maps)
    return np.concatenate([outs[c] for c in range(N_CORES)], axis=0)


def _make_runner(nc):
    """Build a cached PJRT executable (same path run_bass_kernel_spmd takes
    under axon, via bass2jax) so repeat calls don't re-trace."""
    import jax
    import jax.numpy as jnp
    from jax.sharding import Mesh, PartitionSpec
    from jax.experimental.shard_map import shard_map
    from concourse import bass2jax

    bass2jax.install_neuronx_cc_hook()
    partition_name = (nc.partition_id_tensor.name
                      if nc.partition_id_tensor else None)
    in_names = []
    out_names = []
    out_avals = []
    for alloc in nc.m.functions[0].allocations:
        if not isinstance(alloc, mybir.MemoryLocationSet):
            continue
        name = alloc.memorylocations[0].name
        if alloc.kind == "ExternalInput":
            if name != partition_name:
                in_names.append(name)
        elif alloc.kind == "ExternalOutput":
            shape = tuple(alloc.tensor_shape)
            dtype = mybir.dt.np(alloc.dtype)
            out_names.append(name)
            out_avals.append(jax.core.ShapedArray(shape, dtype))
    n_params = len(in_names)
    all_names = in_names + out_names
    if partition_name is not None:
        all_names = all_names + [partition_name]

    def _body(*args):
        operands = list(args)
        if partition_name is not None:
            operands.append(bass2jax.partition_id_tensor())
        outs = bass2jax._bass_exec_p.bind(
            *operands,
            out_avals=tuple(out_avals),
            in_names=tuple(all_names),
            out_names=tuple(out_names),
            lowering_input_output_aliases=(),
            sim_require_finite=True,
            sim_require_nnan=True,
            nc=nc,
        )
        return tuple(outs)

    devices = jax.devices()[:N_CORES]
    mesh = Mesh(np.asarray(devices), ("core",))
    n_outs = len(out_names)
    sharded = jax.jit(
        shard_map(_body, mesh=mesh,
                  in_specs=(PartitionSpec("core"),) * (n_params + n_outs),
                  out_specs=(PartitionSpec("core"),) * n_outs,
                  check_rep=False),
        donate_argnums=tuple(range(n_params, n_params + n_outs)),
        keep_unused=True,
    )
    sharded_nodonate = jax.jit(
        shard_map(_body, mesh=mesh,
                  in_specs=(PartitionSpec("core"),) * (n_params + n_outs),
                  out_specs=(PartitionSpec("core"),) * n_outs,
                  check_rep=False),
        keep_unused=True,
    )

    def run(in_maps):
        concat_in = [
            np.concatenate([np.asarray(in_maps[c][n]) for c in range(N_CORES)],
                           axis=0)
            for n in in_names
        ]
        concat_zeros = [
            np.zeros((N_CORES * a.shape[0], *a.shape[1:]), a.dtype)
            for a in out_avals
        ]
        out_arrs = sharded(*concat_in, *concat_zeros)
        i = out_names.index("out")
        full = np.asarray(out_arrs[i]).reshape(N_CORES, *out_avals[i].shape)
        return [full[c] for c in range(N_CORES)]

    def stage(in_maps):
        """device_put inputs once; return a dispatch closure for timing."""
        from jax.sharding import NamedSharding
        sh = NamedSharding(mesh, PartitionSpec("core"))
        concat_in = [
            jax.device_put(np.concatenate(
                [np.asarray(in_maps[c][n]) for c in range(N_CORES)], axis=0), sh)
            for n in in_names
        ]
        concat_zeros = [
            jax.device_put(
                np.zeros((N_CORES * a.shape[0], *a.shape[1:]), a.dtype), sh)
            for a in out_avals
        ]

        def dispatch():
            return sharded_nodonate(*concat_in, *concat_zeros)

        return dispatch

    run.stage = stage
    return run


# revision 33
# speedup vs baseline: 1.1607x; 1.1238x over previous
"""Trainium2 Bass kernel for a binarized (1w/1a) BasicBlock — fp8 DoubleRow.

    a1 = sign(x);  y1 = BN(conv3x3(a1, binarize(w1))) + x;  x1 = maxout(y1)
    a2 = sign(x1); y2 = BN(conv3x3(a2, binarize(w2))) + x1; out = maxout(y2)

Data-parallel over batch (4 samples/core, 8 cores); exact binary math:
activations are +-1 (fp8e4, exact), weights are sign(+-1) fp8; each conv is
9 DoubleRow matmuls per (chunk, cout-block), contracting all 256 input
channels at once over contiguous padded-row runs (pad columns land in
unused psum columns).  conv_true = alpha_a*alpha[o]*(BB + q[o]*S1) with
q = beta/alpha; S1 (3x3 box of the channel sum) comes from 3 more DoubleRow
ones-matmuls (folding the kh taps) + 2 shifted adds.  The per-channel scale
folds into BN exactly by scaling BN_EPS per channel.  Batch-stat BN uses an
AllGather of per-core (count, mean, M2) triplets + bn_aggr on every core.

Scheduling (v2): sample-outer padded activation layout so the conv stream
starts as soon as the first sample pair is signed; one early warmup
AllGather absorbs the CC-ring establishment; apply1 only computes
t = BN(z)+x (into the dead cv space) and sign(t) -> a2, deferring the
maxout multiply (x1 = t*coef) into the conv2 window; round-2 conv runs
cout-block-major with a per-block stats AllGather so the second collective
overlaps conv2 and apply2(block0).
"""

import numpy as np
import ml_dtypes

import concourse.bass as bass
import concourse.bacc as bacc
import concourse.mybir as mybir
import concourse.tile as tile

N_CORES = 8
B, C, H, W = 32, 256, 28, 28
BPC = B // N_CORES            # samples per core
NBLK = 2                      # channel blocks of 128
HPAD, WPAD = 30, 30           # padded image in SBUF
PIX = H * W                   # 784
PPIX = HPAD * WPAD            # 900
SPLANE = 2 * PPIX             # both channel planes of one sample: 1800
NCHUNK = 2 * BPC              # 8 chunks of (sample, half-image)
HHALF = H // 2                # 14
CHUNK = HHALF * W             # 392 dense output elems per chunk
RUN = HHALF * WPAD            # 420: rhs run length / psum width per chunk
BN_EPS = 1e-5
NPRM = 24
GUARD = 32                    # fp8 guard elems around merged activation tile
SPAN = 4 * CHUNK              # 1568-wide apply spans (2 samples)
F32 = mybir.dt.float32
BF16 = mybir.dt.bfloat16
FP8 = mybir.dt.float8e4
AF = mybir.ActivationFunctionType
ALU = mybir.AluOpType
DR = mybir.MatmulPerfMode.DoubleRow


def _evac(nc, sc, ps, s1, sums, sumsqs, cv, prm, pcol, ci, oblk):
    """z = q[o]*S1 + BB from PSUM (strided: skip pad cols).  Stats come for
    free: the STT accumulates sum(z) on DVE; a Square pass on the otherwise
    idle ScalarE accumulates sum(z^2)."""
    psv = ps[:].rearrange("p (h w) -> p h w", h=HHALF)[:, :, 1:1 + W]
    s1v = s1[:].rearrange("p (h w) -> p h w", h=H)[
        :, (ci % 2) * HHALF:(ci % 2) * HHALF + HHALF, :]
    cvc = cv[oblk][:, ci * CHUNK:(ci + 1) * CHUNK]
    nc.vector.scalar_tensor_tensor(
        cvc.rearrange("p (h w) -> p h w", h=HHALF), s1v,
        prm[:, pcol['q'] + oblk:pcol['q'] + oblk + 1], psv,
        op0=ALU.mult, op1=ALU.add,
        accum_out=sums[:, oblk * NCHUNK + ci:oblk * NCHUNK + ci + 1])
    sqj = sc.tile([128, CHUNK], F32, tag="sqj", name="sqj", bufs=2)
    nc.scalar.activation(
        sqj[:], cvc, AF.Square,
        accum_out=sumsqs[:, oblk * NCHUNK + ci:oblk * NCHUNK + ci + 1])


def _s1_sample(nc, sc, psum, rnd, rhs_ap, ones3, b):
    """S1 (3x3 box of channel sums) for one sample: 3 kh-folding DoubleRow
    ones-matmuls per half + 2 shifted W-direction adds."""
    hs = sc.tile([128, 2 * RUN], F32, tag="hs", name="hs", bufs=2)
    for half in range(2):
        h0 = half * HHALF
        ps2 = psum.tile([128, RUN], F32, tag="ps2", name=f"ps2_{rnd}",
                        bufs=2)
        for kh in range(3):
            nc.tensor.matmul(ps2[:], ones3,
                             rhs_ap(b * SPLANE + (h0 + kh) * WPAD),
                             start=(kh == 0), stop=(kh == 2), perf_mode=DR)
        nc.scalar.copy(hs[:, half * RUN:half * RUN + RUN], ps2[:])
    hsv = hs[:].rearrange("p (h w) -> p h w", h=H)
    w3 = sc.tile([128, H * W], F32, tag="w3", name="w3", bufs=2)
    w3v = w3[:].rearrange("p (h w) -> p h w", h=H)
    nc.vector.tensor_add(w3v, hsv[:, :, 0:W], hsv[:, :, 1:1 + W])
    s1 = sc.tile([128, H * W], F32, tag="s1", name="s1", bufs=BPC)
    s1v = s1[:].rearrange("p (h w) -> p h w", h=H)
    nc.vector.tensor_add(s1v, w3v, hsv[:, :, 2:2 + W])
    return s1


def _chunk_mms(nc, pools, rnd, rhs_ap, wv, s1s, cv, prm, pcol, sums, sumsqs,
               ci, oblk):
    sbuf, psum, sc, dram = pools
    b, h0 = ci // 2, (ci % 2) * HHALF
    ps = psum.tile([128, RUN], F32, tag="ps", name=f"ps{rnd}", bufs=6)
    for k9 in range(9):
        kh, kw = k9 // 3, k9 % 3
        nc.tensor.matmul(
            ps[:], wv[:, k9, :, oblk * 128:(oblk + 1) * 128],
            rhs_ap(b * SPLANE + (h0 + kh) * WPAD + kw - 1),
            start=(k9 == 0), stop=(k9 == 8), perf_mode=DR)
    _evac(nc, sc, ps, s1s[b], sums, sumsqs, cv, prm, pcol, ci, oblk)


def _local_pair(nc, sums, sumsqs, oblk, dest, col):
    """Raw (sum, sumsq) over this oblk's 8 chunks -> dest[:, col:col+2].
    Only two reduces sit before the AllGather trigger."""
    nc.vector.reduce_sum(dest[:, col:col + 1],
                         sums[:, oblk * NCHUNK:(oblk + 1) * NCHUNK],
                         axis=mybir.AxisListType.X)
    nc.vector.reduce_sum(dest[:, col + 1:col + 2],
                         sumsqs[:, oblk * NCHUNK:(oblk + 1) * NCHUNK],
                         axis=mybir.AxisListType.X)


def _ag_start(nc, dram, rnd, pair, npair, tag):
    """DMA the local raw sums to DRAM and trigger the AllGather."""
    b_d = dram.tile([128, 2 * npair], F32, name=f"bd{rnd}{tag}")
    g_d = dram.tile([N_CORES, 128, 2 * npair], F32, name=f"gd{rnd}{tag}")
    nc.gpsimd.dma_start(b_d[:], pair[:])
    nc.gpsimd.collective_compute(
        "AllGather", ALU.bypass,
        replica_groups=[list(range(N_CORES))],
        ins=[b_d.opt()], outs=[g_d.opt()])
    return g_d


def _ag_gather(nc, sbuf, rnd, g_d, npair, tag):
    """Gather back r-major (contiguous per-rank j-tuples -> one fast DMA);
    the per-stat reduction later uses a strided AP."""
    gst = sbuf.tile([128, 2 * npair * N_CORES], F32, name=f"gst{rnd}{tag}")
    nc.sync.dma_start(
        gst[:].rearrange("p (r j) -> p r j", r=N_CORES),
        g_d[:].rearrange("r p j -> p r j"))
    return gst


def _ag_finish(nc, sbuf, rnd, gst, prm, pcol, oblk, jcol, tag):
    """Global mean/var from raw sums; per-channel scale/shift columns."""
    cnt = float(N_CORES * NCHUNK * CHUNK)
    mean = sbuf.tile([128, 1], F32, name=f"mean{rnd}{tag}")
    ms = sbuf.tile([128, 1], F32, name=f"ms{rnd}{tag}")
    m2 = sbuf.tile([128, 1], F32, name=f"m2{rnd}{tag}")
    var = sbuf.tile([128, 1], F32, name=f"var{rnd}{tag}")
    sd = sbuf.tile([128, 1], F32, name=f"sd{rnd}{tag}")
    inv = sbuf.tile([128, 1], F32, name=f"inv{rnd}{tag}")
    scale = sbuf.tile([128, 1], F32, name=f"scale{rnd}{tag}")
    tmp = sbuf.tile([128, 1], F32, name=f"tmp{rnd}{tag}")
    shift = sbuf.tile([128, 1], F32, name=f"shift{rnd}{tag}")
    gv = gst[:].rearrange("p (r j) -> p j r", r=N_CORES)
    nc.vector.reduce_sum(tmp[:], gv[:, jcol], axis=mybir.AxisListType.X)
    nc.vector.tensor_scalar_mul(mean[:], tmp[:], 1.0 / cnt)
    nc.vector.reduce_sum(tmp[:], gv[:, jcol + 1], axis=mybir.AxisListType.X)
    nc.vector.tensor_scalar_mul(ms[:], tmp[:], 1.0 / cnt)
    nc.vector.tensor_mul(m2[:], mean[:], mean[:])
    nc.vector.tensor_sub(var[:], ms[:], m2[:])
    nc.scalar.activation(sd[:], var[:], AF.Sqrt,
                         bias=prm[:, pcol['eps'] + oblk:
                                  pcol['eps'] + oblk + 1],
                         scale=1.0)
    nc.vector.reciprocal(inv[:], sd[:])
    nc.vector.tensor_mul(scale[:], inv[:],
                         prm[:, pcol['g'] + oblk:pcol['g'] + oblk + 1])
    nc.vector.tensor_mul(tmp[:], mean[:], scale[:])
    nc.vector.tensor_sub(shift[:],
                         prm[:, pcol['b'] + oblk:pcol['b'] + oblk + 1],
                         tmp[:])
    return scale, shift


def build():
    nc = bacc.Bacc("TRN2", target_bir_lowering=False, debug=False,
                   enable_asserts=True, num_devices=N_CORES)
    x_d = nc.dram_tensor("x", [BPC, C, H, W], F32, kind="ExternalInput")
    w1_d = nc.dram_tensor("w1t", [9, NBLK, 128, 256], FP8,
                          kind="ExternalInput")
    w2_d = nc.dram_tensor("w2t", [9, NBLK, 128, 256], FP8,
                          kind="ExternalInput")
    prm_d = nc.dram_tensor("prm", [128, NPRM], F32, kind="ExternalInput")
    out_d = nc.dram_tensor("out", [BPC, C, H, W], F32, kind="ExternalOutput")

    with tile.TileContext(nc) as tc:
        with (
            tc.tile_pool(name="sbuf", bufs=1) as sbuf,
            tc.tile_pool(name="psum", bufs=6, space="PSUM") as psum,
            tc.tile_pool(name="sc", bufs=2) as sc,
            tc.tile_pool(name="dram", bufs=1, space="DRAM") as dram,
        ):
            pools = (sbuf, psum, sc, dram)
            # ---- warmup collective first: its trigger starts the ~40us
            # CC-ring establishment clock, which runs while conv1 computes ----
            wu_s = sbuf.tile([1, 16], F32, name="wu_s")
            nc.vector.memset(wu_s[:], 0.0)
            wu_i = dram.tile([1, 16], F32, name="wu_i")
            wu_o = dram.tile([N_CORES, 16], F32, name="wu_o")
            nc.gpsimd.dma_start(wu_i[:], wu_s[:])
            nc.gpsimd.collective_compute(
                "AllGather", ALU.bypass,
                replica_groups=[list(range(N_CORES))],
                ins=[wu_i.opt()], outs=[wu_o.opt()])

            w1sb = sbuf.tile([128, 9 * NBLK * 256], FP8, name="w1sb")
            w2sb = sbuf.tile([128, 9 * NBLK * 256], FP8, name="w2sb")
            prm = sbuf.tile([128, NPRM], F32, name="prm")
            onesb = sbuf.tile([128, 256], FP8, name="onesb")
            nc.vector.memset(onesb[:], 1.0)
            xres = [sbuf.tile([128, BPC * PIX], F32, name=f"xres{i}")
                    for i in range(NBLK)]
            a1p = sbuf.tile([128, GUARD + BPC * SPLANE + GUARD], FP8,
                            name="a1p")
            a2p = sbuf.tile([128, GUARD + BPC * SPLANE + GUARD], FP8,
                            name="a2p")
            cv = [sbuf.tile([128, BPC * PIX], F32, name=f"cv{i}")
                  for i in range(NBLK)]
            xt = [sbuf.tile([128, BPC * PIX], F32, name=f"xt{i}")
                  for i in range(NBLK)]
            cv2 = [sbuf.tile([128, BPC * PIX], BF16, name=f"cv2{i}")
                   for i in range(NBLK)]
            xr2 = [sbuf.tile([128, BPC * PIX], BF16, name=f"xr2{i}")
                   for i in range(NBLK)]

            a1u = a1p[:].bitcast(mybir.dt.uint32)
            half_u = (GUARD + 2 * SPLANE) // 4
            nc.vector.memset(a1u[:, 0:half_u], 0)
            nc.vector.memset(a1u[:, half_u:], 0)
            xv = x_d[:].rearrange("b c h w -> c b (h w)")
            for b in range(BPC):
                for i in range(NBLK):
                    nc.sync.dma_start(
                        xres[i][:, b * PIX:(b + 1) * PIX],
                        xv[i * 128:(i + 1) * 128, b])
            # w1 split across three DMA queues so no single 8us transfer
            # gates the first matmul
            w1v = w1sb[:].rearrange("p (k i o) -> p k i o", k=9, i=NBLK)
            w1dv = w1_d[:].rearrange("k i p o -> p k i o")
            nc.scalar.dma_start(w1v[:, 0:3], w1dv[:, 0:3])
            nc.gpsimd.dma_start(w1v[:, 3:6], w1dv[:, 3:6])
            nc.sync.dma_start(w1v[:, 6:9], w1dv[:, 6:9])
            nc.sync.dma_start(prm[:], prm_d[:])
            nc.gpsimd.memset(a2p[:].bitcast(mybir.dt.uint32), 0)
            nc.sync.dma_start(
                w2sb[:].rearrange("p (k i o) -> p k i o", k=9, i=NBLK),
                w2_d[:].rearrange("k i p o -> p k i o"))

            # a1 = sign(x) into the padded interior, sample-outer layout so
            # sample b's conv work depends only on sample b's signs
            a1v = a1p[:, GUARD:GUARD + BPC * SPLANE].rearrange(
                "p (b i h w) -> p b i h w", b=BPC, i=2, h=HPAD, w=WPAD)
            xrvs = [xres[i][:].rearrange("p (b h w) -> p b h w", b=BPC, h=H)
                    for i in range(NBLK)]
            for b in range(BPC):
                for i in range(NBLK):
                    nc.scalar.activation(a1v[:, b, i, 1:1 + H, 1:1 + W],
                                         xrvs[i][:, b], AF.Sign)

            pcol1 = {'g': 0, 'b': 2, 'hp': 4, 'hs': 6, 'eps': 16, 'q': 20}
            pcol2 = {'g': 8, 'b': 10, 'hp': 12, 'hs': 14, 'eps': 18, 'q': 22}

            def mk_rhs(apad):
                t448 = apad[:, 0:SPLANE].rearrange(
                    "p (i n) -> p i n", i=2)[:, :, 0:RUN]

                def rhs_ap(off):
                    return bass.AP(t448.tensor, GUARD + off, t448.ap)
                return rhs_ap

            rhs1 = mk_rhs(a1p)
            rhs2 = mk_rhs(a2p)
            ones3 = onesb[:].rearrange("p (i o) -> p i o", i=2)
            wv1 = w1sb[:].rearrange("p (k i o) -> p k i o", k=9, i=NBLK)
            wv2 = w2sb[:].rearrange("p (k i o) -> p k i o", k=9, i=NBLK)

            a2iv = a2p[:, GUARD:GUARD + BPC * SPLANE].rearrange(
                "p (sp b2 i h w) -> p sp b2 i h w", sp=2, b2=2, i=2,
                h=HPAD, w=WPAD)
            ov = out_d[:].rearrange("(sp b2) c h w -> c sp b2 (h w)", sp=2)

            # ================= round 1: conv + stats =================
            sums1 = sbuf.tile([128, NCHUNK * NBLK], F32, name="sums1")
            sumsqs1 = sbuf.tile([128, NCHUNK * NBLK], F32, name="sumsqs1")
            s1s = {}
            for pair in range(2):
                for b in (2 * pair, 2 * pair + 1):
                    s1s[b] = _s1_sample(nc, sc, psum, 1, rhs1, ones3, b)
                for ci in range(4 * pair, 4 * pair + 4):
                    for oblk in range(NBLK):
                        _chunk_mms(nc, pools, 1, rhs1, wv1, s1s, cv, prm,
                                   pcol1, sums1, sumsqs1, ci, oblk)
            pair1 = sbuf.tile([128, NBLK * 2], F32, name="pair1")
            for oblk in range(NBLK):
                _local_pair(nc, sums1, sumsqs1, oblk, pair1, oblk * 2)
            g1_d = _ag_start(nc, dram, 1, pair1, NBLK, "")
            gst1 = _ag_gather(nc, sbuf, 1, g1_d, NBLK, "")
            sc1 = [_ag_finish(nc, sbuf, 1, gst1, prm, pcol1, oblk,
                              oblk * 2, f"{oblk}") for oblk in range(NBLK)]

            # ---- apply1 (light): t = scale*z + shift + x into xt;
            # a2 = sign(t).  The maxout multiply is deferred. ----
            for sp in range(2):
                for oblk in range(NBLK):
                    cvs = cv[oblk][:, sp * SPAN:(sp + 1) * SPAN]
                    xrs = xres[oblk][:, sp * SPAN:(sp + 1) * SPAN]
                    xts = xt[oblk][:, sp * SPAN:(sp + 1) * SPAN]
                    scale, shift = sc1[oblk]
                    nc.vector.scalar_tensor_tensor(
                        xts, cvs, scale[:], xrs, op0=ALU.mult, op1=ALU.add)
                    tv = xts.rearrange("p (b2 h w) -> p b2 h w", b2=2, h=H)
                    nc.scalar.activation(a2iv[:, sp, :, oblk, 1:1 + H,
                                              1:1 + W], tv, AF.Sign,
                                         bias=shift[:], scale=1.0)

            # ================= round 2: conv (oblk-major) =================
            sums2 = sbuf.tile([128, NCHUNK * NBLK], F32, name="sums2")
            sumsqs2 = sbuf.tile([128, NCHUNK * NBLK], F32, name="sumsqs2")

            s2s = {}
            for pair in range(2):
                for b in (2 * pair, 2 * pair + 1):
                    s2s[b] = _s1_sample(nc, sc, psum, 2, rhs2, ones3, b)
                for ci in range(4 * pair, 4 * pair + 4):
                    _chunk_mms(nc, pools, 2, rhs2, wv2, s2s, cv2, prm,
                               pcol2, sums2, sumsqs2, ci, 0)
            pair2 = sbuf.tile([128, NBLK * 2], F32, name="pair2")
            _local_pair(nc, sums2, sumsqs2, 0, pair2, 0)

            # x1 = t * (hp*sign(t) + hs): deferred maxout, off the AG2a
            # critical path, overlapped with conv2's oblk-1 sweep
            for sp in range(2):
                for oblk in range(NBLK):
                    xts = xt[oblk][:, sp * SPAN:(sp + 1) * SPAN]
                    xrs = xres[oblk][:, sp * SPAN:(sp + 1) * SPAN]
                    coef = sc.tile([128, SPAN], F32, tag="coef", name="coef",
                                   bufs=2)
                    nc.scalar.activation(
                        coef[:].rearrange("p (b2 h w) -> p b2 h w",
                                          b2=2, h=H),
                        a2iv[:, sp, :, oblk, 1:1 + H, 1:1 + W], AF.Identity,
                        bias=prm[:, pcol1['hs'] + oblk:
                                 pcol1['hs'] + oblk + 1],
                        scale=prm[:, pcol1['hp'] + oblk:
                                  pcol1['hp'] + oblk + 1])
                    nc.vector.scalar_tensor_tensor(
                        xr2[oblk][:, sp * SPAN:(sp + 1) * SPAN], xts,
                        sc1[oblk][1][:], coef[:],
                        op0=ALU.add, op1=ALU.mult)

            for ci in range(NCHUNK):
                _chunk_mms(nc, pools, 2, rhs2, wv2, s2s, cv2, prm,
                           pcol2, sums2, sumsqs2, ci, 1)
            _local_pair(nc, sums2, sumsqs2, 1, pair2, 2)
            g2_d = _ag_start(nc, dram, 2, pair2, NBLK, "")
            gst2 = _ag_gather(nc, sbuf, 2, g2_d, NBLK, "")

            # ---- apply2: one AG for both oblks, bf16 elementwise ----
            sc2 = [_ag_finish(nc, sbuf, 2, gst2, prm, pcol2, oblk,
                              oblk * 2, "ab"[oblk]) for oblk in range(NBLK)]
            for oblk in range(NBLK):
                scale, shift = sc2[oblk]
                for sp in range(2):
                    cvs = cv2[oblk][:, sp * SPAN:(sp + 1) * SPAN]
                    xrs = xr2[oblk][:, sp * SPAN:(sp + 1) * SPAN]
                    t2 = sc.tile([128, SPAN], BF16, tag="u2", name="u2",
                                 bufs=2)
                    nc.vector.scalar_tensor_tensor(
                        t2[:], cvs, scale[:], xrs,
                        op0=ALU.mult, op1=ALU.add)
                    sgt = sc.tile([128, SPAN], BF16, tag="sg", name="sg",
                                  bufs=2)
                    nc.scalar.activation(sgt[:], t2[:], AF.Sign,
                                         bias=shift[:], scale=1.0)
                    coef = sc.tile([128, SPAN], BF16, tag="coef2",
                                   name="coef2", bufs=2)
                    nc.scalar.activation(
                        coef[:], sgt[:], AF.Identity,
                        bias=prm[:, pcol2['hs'] + oblk:
                                 pcol2['hs'] + oblk + 1],
                        scale=prm[:, pcol2['hp'] + oblk:
                                  pcol2['hp'] + oblk + 1])
                    t2h = t2[:].rearrange("p (b2 hw) -> p b2 hw", b2=2)
                    cfh = coef[:].rearrange("p (b2 hw) -> p b2 hw", b2=2)
                    for b2 in range(2):
                        och = sc.tile([128, PIX], F32, tag="och",
                                      name="och", bufs=3)
                        nc.vector.scalar_tensor_tensor(
                            och[:], t2h[:, b2], shift[:], cfh[:, b2],
                            op0=ALU.add, op1=ALU.mult)
                        nc.sync.dma_start(
                            ov[oblk * 128:oblk * 128 + 128, sp, b2],
                            och[:])

    nc.compile()
    return nc


def _prep_weight(w):
    """(O,I,3,3) fp32 -> sign lhsT (9, iblk, 128, 256) fp8 (+-1, exact),
    plus per-output-channel alpha, beta (float64)."""
    w = w.astype(np.float64)
    beta = w.mean(axis=(1, 2, 3))
    alpha = np.sqrt(((w - beta[:, None, None, None]) ** 2)
                    .mean(axis=(1, 2, 3)))
    s = np.sign(w - beta[:, None, None, None]).astype(np.float32)
    wt = s.transpose(2, 3, 1, 0).reshape(9, C, C)   # (k9, i, o)
    wt = wt.reshape(9, NBLK, 128, C)                # (k9, iblk, i, o)
    return wt.astype(ml_dtypes.float8_e4m3), alpha, beta


def make_in_maps(inputs):
    x = np.asarray(inputs['x'], np.float32)
    aa1 = float(np.asarray(inputs['alpha_a1']).reshape(-1)[0])
    aa2 = float(np.asarray(inputs['alpha_a2']).reshape(-1)[0])
    w1t, al1, be1 = _prep_weight(np.asarray(inputs['w1'], np.float32))
    w2t, al2, be2 = _prep_weight(np.asarray(inputs['w2'], np.float32))
    prm = np.zeros((128, NPRM), np.float32)
    f1 = 1.0 / (aa1 * al1)      # z scale relative to the true conv output
    f2 = 1.0 / (aa2 * al2)
    p1 = np.asarray(inputs['pos1'], np.float64)
    n1 = np.asarray(inputs['neg1'], np.float64)
    p2 = np.asarray(inputs['pos2'], np.float64)
    n2 = np.asarray(inputs['neg2'], np.float64)
    cols = ((0, np.asarray(inputs['g1'], np.float64)),
            (2, np.asarray(inputs['b1'], np.float64)),
            (4, (p1 - n1) / 2),
            (6, (p1 + n1) / 2),
            (8, np.asarray(inputs['g2'], np.float64)),
            (10, np.asarray(inputs['b2'], np.float64)),
            (12, (p2 - n2) / 2),
            (14, (p2 + n2) / 2),
            (16, BN_EPS * f1 * f1),
            (18, BN_EPS * f2 * f2),
            (20, be1 / al1),
            (22, be2 / al2))
    for base, arr in cols:
        prm[:, base] = arr[:128]
        prm[:, base + 1] = arr[128:]
    in_maps = []
    for c in range(N_CORES):
        in_maps.append({
            'x': np.ascontiguousarray(x[c * BPC:(c + 1) * BPC]),
            'w1t': w1t, 'w2t': w2t, 'prm': prm,
        })
    return in_maps


_CACHE = {}


def kernel(**inputs):
    in_maps = make_in_maps(inputs)
    if 'run' not in _CACHE:
        nc = build()
        _CACHE['nc'] = nc
        _CACHE['run'] = _make_runner(nc)
    outs = _CACHE['run'](in_maps)
    return np.concatenate([outs[c] for c in range(N_CORES)], axis=0)


def _make_runner(nc):
    """Build a cached PJRT executable (same path run_bass_kernel_spmd takes
    under axon, via bass2jax) so repeat calls don't re-trace."""
    import jax
    import jax.numpy as jnp
    from jax.sharding import Mesh, PartitionSpec
    from jax.experimental.shard_map import shard_map
    from concourse import bass2jax

    bass2jax.install_neuronx_cc_hook()
    partition_name = (nc.partition_id_tensor.name
                      if nc.partition_id_tensor else None)
    in_names = []
    out_names = []
    out_avals = []
    for alloc in nc.m.functions[0].allocations:
        if not isinstance(alloc, mybir.MemoryLocationSet):
            continue
        name = alloc.memorylocations[0].name
        if alloc.kind == "ExternalInput":
            if name != partition_name:
                in_names.append(name)
        elif alloc.kind == "ExternalOutput":
            shape = tuple(alloc.tensor_shape)
            dtype = mybir.dt.np(alloc.dtype)
            out_names.append(name)
            out_avals.append(jax.core.ShapedArray(shape, dtype))
    n_params = len(in_names)
    all_names = in_names + out_names
    if partition_name is not None:
        all_names = all_names + [partition_name]

    def _body(*args):
        operands = list(args)
        if partition_name is not None:
            operands.append(bass2jax.partition_id_tensor())
        outs = bass2jax._bass_exec_p.bind(
            *operands,
            out_avals=tuple(out_avals),
            in_names=tuple(all_names),
            out_names=tuple(out_names),
            lowering_input_output_aliases=(),
            sim_require_finite=True,
            sim_require_nnan=True,
            nc=nc,
        )
        return tuple(outs)

    devices = jax.devices()[:N_CORES]
    mesh = Mesh(np.asarray(devices), ("core",))
    n_outs = len(out_names)
    sharded = jax.jit(
        shard_map(_body, mesh=mesh,
                  in_specs=(PartitionSpec("core"),) * (n_params + n_outs),
                  out_specs=(PartitionSpec("core"),) * n_outs,
                  check_rep=False),
        donate_argnums=tuple(range(n_params, n_params + n_outs)),
        keep_unused=True,
    )
    sharded_nodonate = jax.jit(
        shard_map(_body, mesh=mesh,
                  in_specs=(PartitionSpec("core"),) * (n_params + n_outs),
                  out_specs=(PartitionSpec("core"),) * n_outs,
                  check_rep=False),
        keep_unused=True,
    )

    def run(in_maps):
        concat_in = [
            np.concatenate([np.asarray(in_maps[c][n]) for c in range(N_CORES)],
                           axis=0)
            for n in in_names
        ]
        concat_zeros = [
            np.zeros((N_CORES * a.shape[0], *a.shape[1:]), a.dtype)
            for a in out_avals
        ]
        out_arrs = sharded(*concat_in, *concat_zeros)
        i = out_names.index("out")
        full = np.asarray(out_arrs[i]).reshape(N_CORES, *out_avals[i].shape)
        return [full[c] for c in range(N_CORES)]

    def stage(in_maps):
        """device_put inputs once; return a dispatch closure for timing."""
        from jax.sharding import NamedSharding
        sh = NamedSharding(mesh, PartitionSpec("core"))
        concat_in = [
            jax.device_put(np.concatenate(
                [np.asarray(in_maps[c][n]) for c in range(N_CORES)], axis=0), sh)
            for n in in_names
        ]
        concat_zeros = [
            jax.device_put(
                np.zeros((N_CORES * a.shape[0], *a.shape[1:]), a.dtype), sh)
            for a in out_avals
        ]

        def dispatch():
            return sharded_nodonate(*concat_in, *concat_zeros)

        return dispatch

    run.stage = stage
    return run
